# revision 41
# baseline (speedup 1.0000x reference)
"""OnlineTripletLoss Trainium2 kernel (8 NeuronCores, SPMD).

Strategy (label-space mining):
  pos_mask = positive_idxs[:, target_idx] is a column permutation of the raw
  mask. Instead of permuting the 16MB masks, permute the 2MB embedding once:
  g[l] = embedding[inv_target[l]].  Mining for anchor i then runs over label
  axis l with the raw (contiguous) masks:
      d2'[i,l] = C0 + ||e_i - g_l + eps||^2   (expanded, via PE matmul)
      hardest pos: max over l of 2*d2'[i,l] * mp[i,l]       (mp in {0,1})
      hardest neg: max over l of (KB - 2*d2'[i,l]) * nm[i,l] (nm in {0,1})
  Both minings use ONE custom DVE op (PACK_IDX_RMAX_ANT) per side per
  2048-col piece: it rounds the masked value to an integer (2^23 trick),
  packs (value, index) as value*4096 + idx into one f32 and max-reduces.
  C0 = 1024 pins every valid quantized value q into [2048, 4095], so the
  packed f32 lives in [2^23, 2^24) where ulp = 1 -- the winning label
  index IS the accum's low 12 mantissa bits via direct bitcast-AND (no
  +2^23 normalize op). The neg side mines the SAME max op over the
  ACT-negated staging (Copy(-psum + KB), KB = 4000 + 2*C0): min-d2
  becomes max-value, the index still rides positively, and the neg mask
  stays a plain {0,1} byte. Pos packs read PSUM directly (no staging
  copy on the pos path at all), so PSUM is released after ~2 pack times
  and the first pack starts right after piece-0's matmuls.

  Tail: winner rows are fetched by indirect DMA; ap/an/pn are recomputed
  exactly as one Vector subtract plus one ACT Square-with-accum (fusing
  square AND sum, eps riding the ACT bias) per pair, then one ACT sqrt per
  block. Subtracts for block b are queued on Vector AFTER block b+1's
  packs and all ACT tail work after that block's copies, so neither stalls
  the pack stream; relu is fused into one Vector tensor_scalar (add,max).
  (tensor_tensor_reduce would fuse the whole dot product but it wedges the
  exec unit on this runtime -- NRT_EXEC_UNIT_UNRECOVERABLE.)

  Startup: DMA priority order is block-0's critical path (eT row-block,
  gT, cgo/ar2, first mask halves); mask DMAs are split at the piece
  boundary so the first pack only waits on its own half. Piece-0 and
  piece-1 matmuls use separate SBUF copies of eT so the scheduler's
  LDWEIGHTS affinity cannot pull all 8 data matmuls ahead of piece-0's
  rank-2 finishers (which would delay the first PSUM->SBUF copy).

Per core: 512 anchors x 4096 labels, 4 blocks of 128 anchors.
Output per core: [P, 8] = (masked per-anchor loss | validity); host sums
and divides.
"""

import numpy as np
import ml_dtypes

import concourse.bass as bass
import concourse.mybir as mybir
import concourse.tile as tile
from concourse import bacc
from concourse.bass_utils import run_bass_kernel_spmd
import concourse.dve_ops as dve_ops
from concourse.dve_ops import DveOp
from concourse.dve_spec import (Spec, Src0, Src1, Idx, Zero, maxx,
                                lower, _has_src1)
from concourse.dve_spec import C0 as DVE_C0, C1 as DVE_C1, C2 as DVE_C2
from concourse.dve_uop import DveOpSpec

_OPNAME = "PACK_IDX_RMAX_ANT"


def _ref_pack_idx_rmax(in0, in1, s0, s1, imm2):
    t = (in0.astype(np.float32) * in1).astype(np.float32)
    q = np.float32(np.float32(t + s1) - s1)          # round-to-nearest int
    q = q.reshape(q.shape[0], -1)
    p = np.float32(q * np.float32(imm2)
                   + np.arange(q.shape[-1], dtype=np.float32)[None, :])
    mx = np.maximum(np.float32(0.0), p.max(axis=-1, keepdims=True))
    return p.reshape(in0.shape), mx.astype(np.float32)


def register_pack_idx_rmax():
    """Custom DVE op: p[k] = round(in0[k]*in1[k])*imm2 + k,
    accum_out = max(0, row-max(p)).

    One pass fuses the mask multiply, integer quantization (s1 = 2^23
    round trick), and packing of (quantized value, element index) into one
    exactly-representable f32 (imm2 = 4096 shift), max-reduced. Values are
    arranged (C0 offset / KB flip) so every valid q is in [2048, 4095]:
    the packed value then sits in [2^23, 2^24) and the winning index is
    the accum's low 12 mantissa bits by direct bitcast -- masked-out
    entries pack to bare idx < 4096, far below any valid value, which
    doubles as the validity test. s0 is unused.
    """
    if _OPNAME in dve_ops._SUB_OPCODE_FOR_NAME:
        for op in dve_ops.OPS:
            if op.name == _OPNAME:
                return op
    spec = Spec(body=((Src0 * Src1 + DVE_C1) - DVE_C1) * DVE_C2 + Idx,
                accum=maxx, accum_init=Zero, reference=_ref_pack_idx_rmax)
    row = max(dve_ops._SUB_OPCODE_FOR_NAME.values()) + 1
    assert row < 0x20
    shas = {}
    for ver in ("v3", "v4"):
        try:
            s = DveOpSpec(name=_OPNAME, opcode=row, uops=lower(spec, ver=ver),
                          rd1_en=_has_src1(spec))
            shas[ver] = s.sha(ver)
        except Exception:
            pass
    assert shas, "PACK_IDX_RMAX_ANT failed to lower"
    op = DveOp(_OPNAME, spec, subdim=False, uops_sha=shas)
    dve_ops.OPS.append(op)
    dve_ops.CUSTOM_DVE_SPECS[_OPNAME] = spec
    dve_ops._SUB_OPCODE_FOR_NAME[_OPNAME] = row
    return op


B, D = 4096, 128
M = 8              # cores
BL = B // M        # 512 anchors per core
P = 128            # partition block
NB = BL // P       # 4 anchor blocks per core
CH = 512           # matmul chunk (one psum bank of f32)
HB = 2048          # piece width (half row, 4 psum banks)
EPS = 1e-6
C0 = 1024.0        # pins valid pos q = 2*(C0+d2) into [2048, ~2950]
KB = 4000.0 + 2.0 * C0  # neg flip: q = round(KB - 2*d2') in [~3100, 4000]
MARGIN = 1.0

F32 = mybir.dt.float32
BF16 = mybir.dt.bfloat16
U8 = mybir.dt.uint8
U32 = mybir.dt.uint32
TWO23 = float(2.0 ** 23)
PACK = 4096.0
VTH = 1.0e6        # valid rows pack >= 2^23; masked-out rows <= 4095


def build_nc(debug: bool = False):
    pack_op = register_pack_idx_rmax()
    nc = bacc.Bacc("TRN2", target_bir_lowering=False, debug=debug)

    eT = nc.dram_tensor("eT", [P, BL], BF16, kind="ExternalInput")      # -4*e_local^T
    gT = nc.dram_tensor("gT", [P, B], BF16, kind="ExternalInput")       # g^T
    cgo = nc.dram_tensor("cgo", [2, B], BF16, kind="ExternalInput")     # [2*cg ; ones]
    ar2 = nc.dram_tensor("ar2", [2, BL], BF16, kind="ExternalInput")    # [ones ; 2*arow]
    el = nc.dram_tensor("el", [P, NB, D], F32, kind="ExternalInput")    # anchor rows f32
    gfull = nc.dram_tensor("gfull", [B, D], F32, kind="ExternalInput")  # gather source
    mp = nc.dram_tensor("mp", [BL, B], U8, kind="ExternalInput")        # pos mask {0,1}
    wn = nc.dram_tensor("wn", [BL, B], U8, kind="ExternalInput")        # neg mask {0,1}

    outd = nc.dram_tensor("out", [P, 2 * NB], F32, kind="ExternalOutput")

    with tile.TileContext(nc) as tc:
        with (
            tc.tile_pool(name="singles", bufs=1) as singles,
            tc.tile_pool(name="masks", bufs=3) as maskpool,
            tc.tile_pool(name="stage", bufs=2) as stagepool,
            tc.tile_pool(name="psum", bufs=2, space="PSUM") as psumpool,
        ):
            # ---- startup DMAs, strict priority: piece-0 of block 0 first
            # (its matmuls + pos pack gate everything), then piece 1
            eTa_s = singles.tile([P, BL], BF16)    # piece-0 weights
            nc.sync.dma_start(eTa_s[:, 0:P], eT[:, 0:P])
            gT_s = singles.tile([P, B], BF16)
            for c in range(HB // CH):
                cs = slice(c * CH, (c + 1) * CH)
                nc.sync.dma_start(gT_s[:, cs], gT[:, cs])
            cgo_s = singles.tile([2, B], BF16)
            nc.sync.dma_start(cgo_s[:], cgo[:])
            ar2_s = singles.tile([2, BL], BF16)
            nc.sync.dma_start(ar2_s[:], ar2[:])
            # piece-1 matmul feed interleaved with block-0 mask halves in
            # the order the pack stream consumes them
            eTb_s = singles.tile([P, BL], BF16)    # piece-1 weights (copy)
            nc.sync.dma_start(eTb_s[:, 0:P], eT[:, 0:P])
            mp_t = [None] * NB
            wn_t = [None] * NB
            mp_t[0] = maskpool.tile([P, B], U8, tag="mp", name="mp_b0")
            wn_t[0] = maskpool.tile([P, B], U8, tag="wn", name="wn_b0")
            nc.sync.dma_start(mp_t[0][:, 0:HB // 2], mp[0:P, 0:HB // 2])
            nc.sync.dma_start(mp_t[0][:, HB // 2:HB], mp[0:P, HB // 2:HB])
            for c in range(HB // CH, B // CH):
                cs = slice(c * CH, (c + 1) * CH)
                nc.sync.dma_start(gT_s[:, cs], gT[:, cs])
            nc.sync.dma_start(mp_t[0][:, HB:B], mp[0:P, HB:B])
            nc.sync.dma_start(wn_t[0][:, 0:HB], wn[0:P, 0:HB])
            nc.sync.dma_start(wn_t[0][:, HB:B], wn[0:P, HB:B])
            # non-critical startup loads
            for b in range(1, NB):
                bs = slice(b * P, (b + 1) * P)
                nc.sync.dma_start(eTa_s[:, bs], eT[:, bs])
                nc.sync.dma_start(eTb_s[:, bs], eT[:, bs])

            # ACT table warm-up in the DMA shadow (Square/Sqrt used in the
            # tail; Copy is table-free)
            warm = singles.tile([P, 1], F32)
            nc.vector.memset(warm[:], 1.0)
            nc.scalar.activation(warm[:], warm[:],
                                 mybir.ActivationFunctionType.Square)
            nc.scalar.activation(warm[:], warm[:],
                                 mybir.ActivationFunctionType.Sqrt)

            eps_b = singles.tile([P, 1], F32)
            nc.vector.memset(eps_b[:], EPS)
            m4095 = singles.tile([P, 1], U32)
            nc.vector.memset(m4095[:], 4095)

            el_all = singles.tile([P, NB, D], F32)
            nc.scalar.dma_start(el_all[:], el[:])
            acc2 = singles.tile([P, NB, 4], F32)  # per-piece accums [p0 p1 n0 n1]
            acc_s = singles.tile([P, 2], F32)     # block-0 sub-piece accums
            accP = singles.tile([P, NB], F32)     # merged packed accums
            accN = singles.tile([P, NB], F32)
            idx_pn = singles.tile([P, NB, 2], U32)
            pn_all = singles.tile([P, NB, 2, D], F32)
            v = singles.tile([P, B], F32)         # pack elementwise out (unused)
            dif = singles.tile([P, NB, 3, D], F32)
            sq = singles.tile([P, NB, 3, D], F32)
            rt2 = singles.tile([P, NB, 3], F32)   # squared distances
            rt = singles.tile([P, NB, 3], F32)    # distances
            outt = singles.tile([P, 2 * NB], F32)

            def emit_block(b):
                rs = b * P
                if b + 1 < NB:
                    # prefetch next block's masks, piece-halves first
                    nrs = (b + 1) * P
                    mp_t[b + 1] = maskpool.tile([P, B], U8, tag="mp",
                                                name=f"mp_b{b + 1}")
                    wn_t[b + 1] = maskpool.tile([P, B], U8, tag="wn",
                                                name=f"wn_b{b + 1}")
                    for h in range(2):
                        hs = slice(h * HB, (h + 1) * HB)
                        nc.sync.dma_start(mp_t[b + 1][:, hs], mp[nrs:nrs + P, hs])
                        nc.sync.dma_start(wn_t[b + 1][:, hs], wn[nrs:nrs + P, hs])

                dng = [None, None]
                for pi, lhs in ((0, eTa_s), (1, eTb_s)):
                    hs0 = pi * HB
                    psum = psumpool.tile([P, HB], F32, tag="psum",
                                         name=f"ps_{b}_{pi}")
                    for c in range(HB // CH):
                        gs = slice(hs0 + c * CH, hs0 + (c + 1) * CH)
                        ps = slice(c * CH, (c + 1) * CH)
                        nc.tensor.matmul(
                            psum[:, ps], lhsT=lhs[:, rs:rs + P],
                            rhs=gT_s[:, gs], start=True, stop=False,
                        )
                    for c in range(HB // CH):
                        gs = slice(hs0 + c * CH, hs0 + (c + 1) * CH)
                        ps = slice(c * CH, (c + 1) * CH)
                        nc.tensor.matmul(
                            psum[:, ps], lhsT=ar2_s[:, rs:rs + P],
                            rhs=cgo_s[:, gs], start=False, stop=True,
                        )
                    # neg staging = flipped values; pos packs read PSUM raw
                    dng[pi] = stagepool.tile([P, HB], F32, tag="dng",
                                             name=f"dng_{b}_{pi}")
                    nc.scalar.activation(
                        dng[pi][:], psum[:, 0:HB],
                        mybir.ActivationFunctionType.Copy,
                        bias=KB, scale=-1.0)

                    # hardest-pos pack straight off PSUM (releases it after
                    # one pack; the ACT flip above is the only other reader)
                    nc.vector._custom_dve(
                        pack_op, out=v[:, hs0:hs0 + HB], in0=psum[:, 0:HB],
                        in1=mp_t[b][:, hs0:hs0 + HB], s0=0.0, s1=TWO23,
                        imm2=PACK, accum_out=acc2[:, b, pi:pi + 1])

                # pos decode right after the pos packs: the p-gather then
                # overlaps both neg packs (the ~2.6us DGE dispatch latency
                # means later would miss the window on the last block)
                nc.vector.tensor_scalar(
                    acc2[:, b, 1:2], acc2[:, b, 1:2], 2048.0,
                    scalar2=None, op0=mybir.AluOpType.add)
                nc.vector.tensor_tensor(
                    out=accP[:, b:b + 1], in0=acc2[:, b, 0:1],
                    in1=acc2[:, b, 1:2], op=mybir.AluOpType.max)
                nc.vector.tensor_tensor(
                    out=idx_pn[:, b, 0:1], in0=accP[:, b:b + 1].bitcast(U32),
                    in1=m4095[:, 0:1], op=mybir.AluOpType.bitwise_and)
                nc.gpsimd.indirect_dma_start(
                    out=pn_all[:, b, 0, :], out_offset=None, in_=gfull[:],
                    in_offset=bass.IndirectOffsetOnAxis(
                        ap=idx_pn[:, b, 0:1], axis=0),
                )
                nc.vector._custom_dve(
                    pack_op, out=v[:, 0:HB], in0=dng[0][:],
                    in1=wn_t[b][:, 0:HB], s0=0.0, s1=TWO23,
                    imm2=PACK, accum_out=acc2[:, b, 2:3])
                nc.vector._custom_dve(
                    pack_op, out=v[:, HB:B], in0=dng[1][:],
                    in1=wn_t[b][:, HB:B], s0=0.0, s1=TWO23,
                    imm2=PACK, accum_out=acc2[:, b, 3:4])
                nc.vector.tensor_scalar(
                    acc2[:, b, 3:4], acc2[:, b, 3:4], 2048.0,
                    scalar2=None, op0=mybir.AluOpType.add)
                nc.vector.tensor_tensor(
                    out=accN[:, b:b + 1], in0=acc2[:, b, 2:3],
                    in1=acc2[:, b, 3:4], op=mybir.AluOpType.max)
                nc.vector.tensor_tensor(
                    out=idx_pn[:, b, 1:2], in0=accN[:, b:b + 1].bitcast(U32),
                    in1=m4095[:, 0:1], op=mybir.AluOpType.bitwise_and)
                nc.gpsimd.indirect_dma_start(
                    out=pn_all[:, b, 1, :], out_offset=None, in_=gfull[:],
                    in_offset=bass.IndirectOffsetOnAxis(
                        ap=idx_pn[:, b, 1:2], axis=0),
                )

            def emit_tail_vec(b):
                # exact difference rows (a-p, a-n, p-n)
                nc.vector.tensor_sub(dif[:, b, 0, :], el_all[:, b, :],
                                     pn_all[:, b, 0, :])
                nc.vector.tensor_sub(dif[:, b, 1, :], el_all[:, b, :],
                                     pn_all[:, b, 1, :])
                nc.vector.tensor_sub(dif[:, b, 2, :], pn_all[:, b, 0, :],
                                     pn_all[:, b, 1, :])

            def emit_tail_act(b):
                # squared distance = ACT Square with fused sum-accum,
                # eps rides the bias: sum((dif + eps)^2); then sqrt
                for k in range(3):
                    nc.scalar.activation(
                        sq[:, b, k, :], dif[:, b, k, :],
                        mybir.ActivationFunctionType.Square,
                        bias=eps_b[:, 0:1], scale=1.0,
                        accum_out=rt2[:, b, k:k + 1])
                nc.scalar.activation(rt[:, b, :], rt2[:, b, :],
                                     mybir.ActivationFunctionType.Sqrt)

            # pack stream with tail subtracts lagging one block so Vector
            # never stalls on gathers; ACT tail work queued behind the
            # flips it must not delay
            emit_block(0)
            emit_block(1)
            emit_tail_vec(0)
            emit_block(2)
            emit_tail_act(0)
            emit_tail_vec(1)
            emit_block(3)
            emit_tail_act(1)
            emit_tail_vec(2)
            emit_tail_act(2)
            # validity from the merged accums (invalid rows accumulate low);
            # runs while the last gathers are in flight
            vp = singles.tile([P, NB], F32)
            vn = singles.tile([P, NB], F32)
            nc.vector.tensor_scalar(vp[:], accP[:], VTH, scalar2=None,
                                    op0=mybir.AluOpType.is_gt)
            nc.vector.tensor_scalar(vn[:], accN[:], VTH, scalar2=None,
                                    op0=mybir.AluOpType.is_gt)
            nc.vector.tensor_mul(outt[:, NB:2 * NB], vp[:], vn[:])
            emit_tail_vec(3)
            emit_tail_act(3)

            # loss = max(ap - min(an, pn) + margin, 0) * valid
            mn2 = singles.tile([P, NB], F32)
            nc.vector.tensor_tensor(out=mn2[:], in0=rt[:, :, 1],
                                    in1=rt[:, :, 2], op=mybir.AluOpType.min)
            dff = singles.tile([P, NB], F32)
            nc.vector.tensor_sub(dff[:], rt[:, :, 0], mn2[:])
            lossb = singles.tile([P, NB], F32)
            nc.vector.tensor_scalar(lossb[:], dff[:], MARGIN, scalar2=0.0,
                                    op0=mybir.AluOpType.add,
                                    op1=mybir.AluOpType.max)
            nc.vector.tensor_mul(outt[:, 0:NB], lossb[:], outt[:, NB:2 * NB])
            nc.scalar.dma_start(outd[:], outt[:])

    nc.finalize()
    return nc


def make_in_maps(embedding, target_idx, positive_idxs, negative_idxs):
    e = np.asarray(embedding, np.float32)
    tid = np.asarray(target_idx, np.int64)
    pos = np.asarray(positive_idxs)
    neg = np.asarray(negative_idxs)

    inv = np.empty(B, np.int64)
    inv[tid] = np.arange(B)
    g = np.ascontiguousarray(e[inv])                       # [B, D] f32

    e64 = e.astype(np.float64)
    g64 = g.astype(np.float64)
    sq_a = (e64 * e64).sum(1)
    s_a = e64.sum(1)
    sq_g = (g64 * g64).sum(1)
    s_g = g64.sum(1)

    gT_bf = np.ascontiguousarray(g.T).astype(ml_dtypes.bfloat16)         # [D, B]
    # doubled pipeline so packed quantization is 0.5 d2-units
    cgo_np = np.ones((2, B), np.float32)
    cgo_np[0] = 2.0 * (sq_g - 2.0 * EPS * s_g)
    cgo_bf = cgo_np.astype(ml_dtypes.bfloat16)
    arow_full = np.asarray(
        2.0 * (sq_a + 2.0 * EPS * s_a + D * EPS * EPS + C0), np.float32)

    in_maps = []
    for m in range(M):
        r = slice(m * BL, (m + 1) * BL)
        # [P, NB(, D)] layouts: block index on the free axis
        el3 = np.ascontiguousarray(
            e[r].reshape(NB, P, D).transpose(1, 0, 2))
        ar2_np = np.ones((2, BL), np.float32)
        ar2_np[1] = arow_full[r]
        in_maps.append({
            "eT": np.ascontiguousarray((-4.0 * e[r].T)).astype(ml_dtypes.bfloat16),
            "gT": gT_bf,
            "cgo": cgo_bf,
            "ar2": ar2_np.astype(ml_dtypes.bfloat16),
            "el": el3,
            "gfull": g,
            "mp": np.ascontiguousarray(pos[r].astype(np.uint8)),
            "wn": np.ascontiguousarray(neg[r].astype(np.uint8)),
        })
    return in_maps


_NC_CACHE = {}


def kernel(embedding, target_idx, positive_idxs, negative_idxs):
    in_maps = make_in_maps(embedding, target_idx, positive_idxs, negative_idxs)
    if "nc" not in _NC_CACHE:
        _NC_CACHE["nc"] = build_nc(debug=False)
    nc = _NC_CACHE["nc"]
    res = run_bass_kernel_spmd(nc, in_maps, core_ids=list(range(M)))
    total_loss = np.float64(0.0)
    total_valid = np.float64(0.0)
    for r in res.results:
        o = np.asarray(r["out"], np.float64)
        total_loss += o[:, 0:NB].sum()
        total_valid += o[:, NB:2 * NB].sum()
    return np.float32(total_loss / max(total_valid, 1.0))


# revision 42
# speedup vs baseline: 1.0491x; 1.0491x over previous
"""OnlineTripletLoss Trainium2 kernel (8 NeuronCores, SPMD).

Strategy (label-space mining):
  pos_mask = positive_idxs[:, target_idx] is a column permutation of the raw
  mask. Instead of permuting the 16MB masks, permute the 2MB embedding once:
  g[l] = embedding[inv_target[l]].  Mining for anchor i then runs over label
  axis l with the raw (contiguous) masks:
      d2'[i,l] = C0 + ||e_i - g_l + eps||^2   (expanded, via PE matmul)
      hardest pos: max over l of 2*d2'[i,l] * mp[i,l]       (mp in {0,1})
      hardest neg: max over l of (KB - 2*d2'[i,l]) * nm[i,l] (nm in {0,1})
  Both minings use ONE custom DVE op (PACK_IDX_RMAX_ANT) per side per
  2048-col piece: it rounds the masked value to an integer (2^23 trick),
  packs (value, index) as value*4096 + idx into one f32 and max-reduces.
  C0 = 1024 pins every valid quantized value q into [2048, 4095], so the
  packed f32 lives in [2^23, 2^24) where ulp = 1 -- the winning label
  index IS the accum's low 12 mantissa bits via direct bitcast-AND (no
  +2^23 normalize op). The neg side mines the SAME max op over the
  ACT-negated staging (Copy(-psum + KB), KB = 4000 + 2*C0): min-d2
  becomes max-value, the index still rides positively, and the neg mask
  stays a plain {0,1} byte. Pos packs read PSUM directly (no staging
  copy on the pos path at all), so PSUM is released after ~2 pack times
  and the first pack starts right after piece-0's matmuls.

  Tail: winner rows are fetched by indirect DMA; ap/an/pn are recomputed
  exactly as one Vector subtract plus one ACT Square-with-accum (fusing
  square AND sum, eps riding the ACT bias) per pair, then one ACT sqrt per
  block. Subtracts for block b are queued on Vector AFTER block b+1's
  packs and all ACT tail work after that block's copies, so neither stalls
  the pack stream; relu is fused into one Vector tensor_scalar (add,max).
  (tensor_tensor_reduce would fuse the whole dot product but it wedges the
  exec unit on this runtime -- NRT_EXEC_UNIT_UNRECOVERABLE.)

  Startup: DMA priority order is block-0's critical path (eT row-block,
  gT, cgo/ar2, first mask halves); mask DMAs are split at the piece
  boundary so the first pack only waits on its own half. Piece-0 and
  piece-1 matmuls use separate SBUF copies of eT so the scheduler's
  LDWEIGHTS affinity cannot pull all 8 data matmuls ahead of piece-0's
  rank-2 finishers (which would delay the first PSUM->SBUF copy).

Per core: 512 anchors x 4096 labels, 4 blocks of 128 anchors.
Output per core: [P, 8] = (masked per-anchor loss | validity); host sums
and divides.
"""

import numpy as np
import ml_dtypes

import concourse.bass as bass
import concourse.mybir as mybir
import concourse.tile as tile
from concourse import bacc
from concourse.bass_utils import run_bass_kernel_spmd
import concourse.dve_ops as dve_ops
from concourse.dve_ops import DveOp
from concourse.dve_spec import (Spec, Src0, Src1, Idx, Zero, maxx,
                                lower, _has_src1)
from concourse.dve_spec import C0 as DVE_C0, C1 as DVE_C1, C2 as DVE_C2
from concourse.dve_uop import DveOpSpec

_OPNAME = "PACK_IDX_RMAX_ANT"


def _ref_pack_idx_rmax(in0, in1, s0, s1, imm2):
    t = (in0.astype(np.float32) * in1).astype(np.float32)
    q = np.float32(np.float32(t + s1) - s1)          # round-to-nearest int
    q = q.reshape(q.shape[0], -1)
    p = np.float32(q * np.float32(imm2)
                   + np.arange(q.shape[-1], dtype=np.float32)[None, :])
    mx = np.maximum(np.float32(0.0), p.max(axis=-1, keepdims=True))
    return p.reshape(in0.shape), mx.astype(np.float32)


def register_pack_idx_rmax():
    """Custom DVE op: p[k] = round(in0[k]*in1[k])*imm2 + k,
    accum_out = max(0, row-max(p)).

    One pass fuses the mask multiply, integer quantization (s1 = 2^23
    round trick), and packing of (quantized value, element index) into one
    exactly-representable f32 (imm2 = 4096 shift), max-reduced. Values are
    arranged (C0 offset / KB flip) so every valid q is in [2048, 4095]:
    the packed value then sits in [2^23, 2^24) and the winning index is
    the accum's low 12 mantissa bits by direct bitcast -- masked-out
    entries pack to bare idx < 4096, far below any valid value, which
    doubles as the validity test. s0 is unused.
    """
    if _OPNAME in dve_ops._SUB_OPCODE_FOR_NAME:
        for op in dve_ops.OPS:
            if op.name == _OPNAME:
                return op
    spec = Spec(body=((Src0 * Src1 + DVE_C1) - DVE_C1) * DVE_C2 + Idx,
                accum=maxx, accum_init=Zero, reference=_ref_pack_idx_rmax)
    row = max(dve_ops._SUB_OPCODE_FOR_NAME.values()) + 1
    assert row < 0x20
    shas = {}
    for ver in ("v3", "v4"):
        try:
            s = DveOpSpec(name=_OPNAME, opcode=row, uops=lower(spec, ver=ver),
                          rd1_en=_has_src1(spec))
            shas[ver] = s.sha(ver)
        except Exception:
            pass
    assert shas, "PACK_IDX_RMAX_ANT failed to lower"
    op = DveOp(_OPNAME, spec, subdim=False, uops_sha=shas)
    dve_ops.OPS.append(op)
    dve_ops.CUSTOM_DVE_SPECS[_OPNAME] = spec
    dve_ops._SUB_OPCODE_FOR_NAME[_OPNAME] = row
    return op


B, D = 4096, 128
M = 8              # cores
BL = B // M        # 512 anchors per core
P = 128            # partition block
NB = BL // P       # 4 anchor blocks per core
CH = 512           # matmul chunk (one psum bank of f32)
HB = 2048          # piece width (half row, 4 psum banks)
EPS = 1e-6
C0 = 1024.0        # pins valid pos q = 2*(C0+d2) into [2048, ~2950]
KB = 4000.0 + 2.0 * C0  # neg flip: q = round(KB - 2*d2') in [~3100, 4000]
MARGIN = 1.0

F32 = mybir.dt.float32
BF16 = mybir.dt.bfloat16
U8 = mybir.dt.uint8
U32 = mybir.dt.uint32
TWO23 = float(2.0 ** 23)
PACK = 4096.0
VTH = 1.0e6        # valid rows pack >= 2^23; masked-out rows <= 4095


def build_nc(debug: bool = False):
    pack_op = register_pack_idx_rmax()
    nc = bacc.Bacc("TRN2", target_bir_lowering=False, debug=debug)

    eT = nc.dram_tensor("eT", [P, BL], BF16, kind="ExternalInput")      # -4*e_local^T
    gT = nc.dram_tensor("gT", [P, B], BF16, kind="ExternalInput")       # g^T
    cgo = nc.dram_tensor("cgo", [2, B], BF16, kind="ExternalInput")     # [2*cg ; ones]
    ar2 = nc.dram_tensor("ar2", [2, BL], BF16, kind="ExternalInput")    # [ones ; 2*arow]
    el = nc.dram_tensor("el", [P, NB, D], F32, kind="ExternalInput")    # anchor rows f32
    gfull = nc.dram_tensor("gfull", [B, D], F32, kind="ExternalInput")  # gather source
    mp = nc.dram_tensor("mp", [BL, B], U8, kind="ExternalInput")        # pos mask {0,1}
    wn = nc.dram_tensor("wn", [BL, B], U8, kind="ExternalInput")        # neg mask {0,1}

    outd = nc.dram_tensor("out", [P, 2 * NB], F32, kind="ExternalOutput")

    with tile.TileContext(nc) as tc:
        with (
            tc.tile_pool(name="singles", bufs=1) as singles,
            tc.tile_pool(name="masks", bufs=3) as maskpool,
            tc.tile_pool(name="stage", bufs=2) as stagepool,
            tc.tile_pool(name="psum", bufs=2, space="PSUM") as psumpool,
        ):
            # ---- startup DMAs, strict priority: piece-0 of block 0 first
            # (its matmuls + pos pack gate everything), then piece 1
            eTa_s = singles.tile([P, BL], BF16)    # piece-0 weights
            nc.sync.dma_start(eTa_s[:, 0:P], eT[:, 0:P])
            gT_s = singles.tile([P, B], BF16)
            for c in range(HB // CH):
                cs = slice(c * CH, (c + 1) * CH)
                nc.sync.dma_start(gT_s[:, cs], gT[:, cs])
            cgo_s = singles.tile([2, B], BF16)
            nc.sync.dma_start(cgo_s[:], cgo[:])
            ar2_s = singles.tile([2, BL], BF16)
            nc.sync.dma_start(ar2_s[:], ar2[:])
            # piece-1 matmul feed interleaved with block-0 mask halves in
            # the order the pack stream consumes them
            eTb_s = singles.tile([P, BL], BF16)    # piece-1 weights (copy)
            nc.sync.dma_start(eTb_s[:, 0:P], eT[:, 0:P])
            mp_t = [None] * NB
            wn_t = [None] * NB
            mp_t[0] = maskpool.tile([P, B], U8, tag="mp", name="mp_b0")
            wn_t[0] = maskpool.tile([P, B], U8, tag="wn", name="wn_b0")
            nc.sync.dma_start(mp_t[0][:, 0:HB], mp[0:P, 0:HB])
            for c in range(HB // CH, B // CH):
                cs = slice(c * CH, (c + 1) * CH)
                nc.sync.dma_start(gT_s[:, cs], gT[:, cs])
            nc.sync.dma_start(mp_t[0][:, HB:B], mp[0:P, HB:B])
            nc.sync.dma_start(wn_t[0][:, 0:HB], wn[0:P, 0:HB])
            nc.sync.dma_start(wn_t[0][:, HB:B], wn[0:P, HB:B])
            # non-critical startup loads
            for b in range(1, NB):
                bs = slice(b * P, (b + 1) * P)
                nc.sync.dma_start(eTa_s[:, bs], eT[:, bs])
                nc.sync.dma_start(eTb_s[:, bs], eT[:, bs])

            # ACT table warm-up in the DMA shadow (Square/Sqrt used in the
            # tail; Copy is table-free)
            warm = singles.tile([P, 1], F32)
            nc.vector.memset(warm[:], 1.0)
            nc.scalar.activation(warm[:], warm[:],
                                 mybir.ActivationFunctionType.Square)
            nc.scalar.activation(warm[:], warm[:],
                                 mybir.ActivationFunctionType.Sqrt)

            eps_b = singles.tile([P, 1], F32)
            nc.vector.memset(eps_b[:], EPS)
            m4095 = singles.tile([P, 1], U32)
            nc.vector.memset(m4095[:], 4095)

            el_all = singles.tile([P, NB, D], F32)
            nc.scalar.dma_start(el_all[:], el[:])
            acc2 = singles.tile([P, NB, 4], F32)  # per-piece accums [p0 p1 n0 n1]
            acc_s = singles.tile([P, 2], F32)     # block-0 sub-piece accums
            accP = singles.tile([P, NB], F32)     # merged packed accums
            accN = singles.tile([P, NB], F32)
            idx_pn = singles.tile([P, NB, 2], U32)
            pn_all = singles.tile([P, NB, 2, D], F32)
            v = singles.tile([P, B], F32)         # pack elementwise out (unused)
            dif = singles.tile([P, NB, 3, D], F32)
            sq = singles.tile([P, NB, 3, D], F32)
            rt2 = singles.tile([P, NB, 3], F32)   # squared distances
            rt = singles.tile([P, NB, 3], F32)    # distances
            outt = singles.tile([P, 2 * NB], F32)

            def emit_block(b):
                rs = b * P
                if b + 1 < NB:
                    # prefetch next block's masks, piece-halves first
                    nrs = (b + 1) * P
                    mp_t[b + 1] = maskpool.tile([P, B], U8, tag="mp",
                                                name=f"mp_b{b + 1}")
                    wn_t[b + 1] = maskpool.tile([P, B], U8, tag="wn",
                                                name=f"wn_b{b + 1}")
                    for h in range(2):
                        hs = slice(h * HB, (h + 1) * HB)
                        nc.sync.dma_start(mp_t[b + 1][:, hs], mp[nrs:nrs + P, hs])
                        nc.sync.dma_start(wn_t[b + 1][:, hs], wn[nrs:nrs + P, hs])

                dng = [None, None]
                for pi, lhs in ((0, eTa_s), (1, eTb_s)):
                    hs0 = pi * HB
                    psum = psumpool.tile([P, HB], F32, tag="psum",
                                         name=f"ps_{b}_{pi}")
                    for c in range(HB // CH):
                        gs = slice(hs0 + c * CH, hs0 + (c + 1) * CH)
                        ps = slice(c * CH, (c + 1) * CH)
                        nc.tensor.matmul(
                            psum[:, ps], lhsT=lhs[:, rs:rs + P],
                            rhs=gT_s[:, gs], start=True, stop=False,
                        )
                    for c in range(HB // CH):
                        gs = slice(hs0 + c * CH, hs0 + (c + 1) * CH)
                        ps = slice(c * CH, (c + 1) * CH)
                        nc.tensor.matmul(
                            psum[:, ps], lhsT=ar2_s[:, rs:rs + P],
                            rhs=cgo_s[:, gs], start=False, stop=True,
                        )
                    # neg staging = flipped values; pos packs read PSUM raw
                    dng[pi] = stagepool.tile([P, HB], F32, tag="dng",
                                             name=f"dng_{b}_{pi}")
                    nc.scalar.activation(
                        dng[pi][:], psum[:, 0:HB],
                        mybir.ActivationFunctionType.Copy,
                        bias=KB, scale=-1.0)

                    # hardest-pos pack straight off PSUM (releases it after
                    # one pack; the ACT flip above is the only other reader)
                    nc.vector._custom_dve(
                        pack_op, out=v[:, hs0:hs0 + HB], in0=psum[:, 0:HB],
                        in1=mp_t[b][:, hs0:hs0 + HB], s0=0.0, s1=TWO23,
                        imm2=PACK, accum_out=acc2[:, b, pi:pi + 1])

                # pos decode right after the pos packs: the p-gather then
                # overlaps both neg packs (the ~2.6us DGE dispatch latency
                # means later would miss the window on the last block)
                nc.vector.tensor_scalar(
                    acc2[:, b, 1:2], acc2[:, b, 1:2], 2048.0,
                    scalar2=None, op0=mybir.AluOpType.add)
                nc.vector.tensor_tensor(
                    out=accP[:, b:b + 1], in0=acc2[:, b, 0:1],
                    in1=acc2[:, b, 1:2], op=mybir.AluOpType.max)
                nc.vector.tensor_tensor(
                    out=idx_pn[:, b, 0:1], in0=accP[:, b:b + 1].bitcast(U32),
                    in1=m4095[:, 0:1], op=mybir.AluOpType.bitwise_and)
                nc.gpsimd.indirect_dma_start(
                    out=pn_all[:, b, 0, :], out_offset=None, in_=gfull[:],
                    in_offset=bass.IndirectOffsetOnAxis(
                        ap=idx_pn[:, b, 0:1], axis=0),
                )
                nc.vector._custom_dve(
                    pack_op, out=v[:, 0:HB], in0=dng[0][:],
                    in1=wn_t[b][:, 0:HB], s0=0.0, s1=TWO23,
                    imm2=PACK, accum_out=acc2[:, b, 2:3])
                nc.vector._custom_dve(
                    pack_op, out=v[:, HB:B], in0=dng[1][:],
                    in1=wn_t[b][:, HB:B], s0=0.0, s1=TWO23,
                    imm2=PACK, accum_out=acc2[:, b, 3:4])
                nc.vector.tensor_scalar(
                    acc2[:, b, 3:4], acc2[:, b, 3:4], 2048.0,
                    scalar2=None, op0=mybir.AluOpType.add)
                nc.vector.tensor_tensor(
                    out=accN[:, b:b + 1], in0=acc2[:, b, 2:3],
                    in1=acc2[:, b, 3:4], op=mybir.AluOpType.max)
                nc.vector.tensor_tensor(
                    out=idx_pn[:, b, 1:2], in0=accN[:, b:b + 1].bitcast(U32),
                    in1=m4095[:, 0:1], op=mybir.AluOpType.bitwise_and)
                nc.gpsimd.indirect_dma_start(
                    out=pn_all[:, b, 1, :], out_offset=None, in_=gfull[:],
                    in_offset=bass.IndirectOffsetOnAxis(
                        ap=idx_pn[:, b, 1:2], axis=0),
                )

            def emit_tail_vec(b):
                # exact difference rows (a-p, a-n, p-n)
                nc.vector.tensor_sub(dif[:, b, 0, :], el_all[:, b, :],
                                     pn_all[:, b, 0, :])
                nc.vector.tensor_sub(dif[:, b, 1, :], el_all[:, b, :],
                                     pn_all[:, b, 1, :])
                nc.vector.tensor_sub(dif[:, b, 2, :], pn_all[:, b, 0, :],
                                     pn_all[:, b, 1, :])

            def emit_tail_act(b):
                # squared distance = ACT Square with fused sum-accum,
                # eps rides the bias: sum((dif + eps)^2); then sqrt
                for k in range(3):
                    nc.scalar.activation(
                        sq[:, b, k, :], dif[:, b, k, :],
                        mybir.ActivationFunctionType.Square,
                        bias=eps_b[:, 0:1], scale=1.0,
                        accum_out=rt2[:, b, k:k + 1])
                nc.scalar.activation(rt[:, b, :], rt2[:, b, :],
                                     mybir.ActivationFunctionType.Sqrt)

            # pack stream with tail subtracts lagging one block so Vector
            # never stalls on gathers; ACT tail work queued behind the
            # flips it must not delay
            emit_block(0)
            emit_block(1)
            emit_tail_vec(0)
            emit_block(2)
            emit_tail_act(0)
            emit_tail_vec(1)
            emit_block(3)
            emit_tail_act(1)
            emit_tail_vec(2)
            emit_tail_act(2)
            # validity from the merged accums (invalid rows accumulate low);
            # runs while the last gathers are in flight
            vp = singles.tile([P, NB], F32)
            vn = singles.tile([P, NB], F32)
            nc.vector.tensor_scalar(vp[:], accP[:], VTH, scalar2=None,
                                    op0=mybir.AluOpType.is_gt)
            nc.vector.tensor_scalar(vn[:], accN[:], VTH, scalar2=None,
                                    op0=mybir.AluOpType.is_gt)
            nc.vector.tensor_mul(outt[:, NB:2 * NB], vp[:], vn[:])
            emit_tail_vec(3)
            emit_tail_act(3)

            # loss = max(ap - min(an, pn) + margin, 0) * valid
            mn2 = singles.tile([P, NB], F32)
            nc.vector.tensor_tensor(out=mn2[:], in0=rt[:, :, 1],
                                    in1=rt[:, :, 2], op=mybir.AluOpType.min)
            dff = singles.tile([P, NB], F32)
            nc.vector.tensor_sub(dff[:], rt[:, :, 0], mn2[:])
            lossb = singles.tile([P, NB], F32)
            nc.vector.tensor_scalar(lossb[:], dff[:], MARGIN, scalar2=0.0,
                                    op0=mybir.AluOpType.add,
                                    op1=mybir.AluOpType.max)
            nc.vector.tensor_mul(outt[:, 0:NB], lossb[:], outt[:, NB:2 * NB])
            nc.scalar.dma_start(outd[:], outt[:])

    nc.finalize()
    return nc


def make_in_maps(embedding, target_idx, positive_idxs, negative_idxs):
    e = np.asarray(embedding, np.float32)
    tid = np.asarray(target_idx, np.int64)
    pos = np.asarray(positive_idxs)
    neg = np.asarray(negative_idxs)

    inv = np.empty(B, np.int64)
    inv[tid] = np.arange(B)
    g = np.ascontiguousarray(e[inv])                       # [B, D] f32

    e64 = e.astype(np.float64)
    g64 = g.astype(np.float64)
    sq_a = (e64 * e64).sum(1)
    s_a = e64.sum(1)
    sq_g = (g64 * g64).sum(1)
    s_g = g64.sum(1)

    gT_bf = np.ascontiguousarray(g.T).astype(ml_dtypes.bfloat16)         # [D, B]
    # doubled pipeline so packed quantization is 0.5 d2-units
    cgo_np = np.ones((2, B), np.float32)
    cgo_np[0] = 2.0 * (sq_g - 2.0 * EPS * s_g)
    cgo_bf = cgo_np.astype(ml_dtypes.bfloat16)
    arow_full = np.asarray(
        2.0 * (sq_a + 2.0 * EPS * s_a + D * EPS * EPS + C0), np.float32)

    in_maps = []
    for m in range(M):
        r = slice(m * BL, (m + 1) * BL)
        # [P, NB(, D)] layouts: block index on the free axis
        el3 = np.ascontiguousarray(
            e[r].reshape(NB, P, D).transpose(1, 0, 2))
        ar2_np = np.ones((2, BL), np.float32)
        ar2_np[1] = arow_full[r]
        in_maps.append({
            "eT": np.ascontiguousarray((-4.0 * e[r].T)).astype(ml_dtypes.bfloat16),
            "gT": gT_bf,
            "cgo": cgo_bf,
            "ar2": ar2_np.astype(ml_dtypes.bfloat16),
            "el": el3,
            "gfull": g,
            "mp": np.ascontiguousarray(pos[r].astype(np.uint8)),
            "wn": np.ascontiguousarray(neg[r].astype(np.uint8)),
        })
    return in_maps


_NC_CACHE = {}


def kernel(embedding, target_idx, positive_idxs, negative_idxs):
    in_maps = make_in_maps(embedding, target_idx, positive_idxs, negative_idxs)
    if "nc" not in _NC_CACHE:
        _NC_CACHE["nc"] = build_nc(debug=False)
    nc = _NC_CACHE["nc"]
    res = run_bass_kernel_spmd(nc, in_maps, core_ids=list(range(M)))
    total_loss = np.float64(0.0)
    total_valid = np.float64(0.0)
    for r in res.results:
        o = np.asarray(r["out"], np.float64)
        total_loss += o[:, 0:NB].sum()
        total_valid += o[:, NB:2 * NB].sum()
    return np.float32(total_loss / max(total_valid, 1.0))


# revision 44
# speedup vs baseline: 1.0669x; 1.0169x over previous
"""OnlineTripletLoss Trainium2 kernel (8 NeuronCores, SPMD).

Strategy (label-space mining):
  pos_mask = positive_idxs[:, target_idx] is a column permutation of the raw
  mask. Instead of permuting the 16MB masks, permute the 2MB embedding once:
  g[l] = embedding[inv_target[l]].  Mining for anchor i then runs over label
  axis l with the raw (contiguous) masks:
      d2'[i,l] = C0 + ||e_i - g_l + eps||^2   (expanded, via PE matmul)
      hardest pos: max over l of 2*d2'[i,l] * mp[i,l]       (mp in {0,1})
      hardest neg: max over l of (KB - 2*d2'[i,l]) * nm[i,l] (nm in {0,1})
  Both minings use ONE custom DVE op (PACK_IDX_RMAX_ANT) per side per
  2048-col piece: it rounds the masked value to an integer (2^23 trick),
  packs (value, index) as value*4096 + idx into one f32 and max-reduces.
  C0 = 1024 pins every valid quantized value q into [2048, 4095], so the
  packed f32 lives in [2^23, 2^24) where ulp = 1 -- the winning label
  index IS the accum's low 12 mantissa bits via direct bitcast-AND (no
  +2^23 normalize op). The neg side mines the SAME max op over the
  ACT-negated staging (Copy(-psum + KB), KB = 4000 + 2*C0): min-d2
  becomes max-value, the index still rides positively, and the neg mask
  stays a plain {0,1} byte. Pos packs read PSUM directly (no staging
  copy on the pos path at all), so PSUM is released after ~2 pack times
  and the first pack starts right after piece-0's matmuls.

  Tail: winner rows are fetched by indirect DMA; ap/an/pn are recomputed
  exactly as one Vector subtract plus one ACT Square-with-accum (fusing
  square AND sum, eps riding the ACT bias) per pair, then one ACT sqrt per
  block. Subtracts for block b are queued on Vector AFTER block b+1's
  packs and all ACT tail work after that block's copies, so neither stalls
  the pack stream; relu is fused into one Vector tensor_scalar (add,max).
  (tensor_tensor_reduce would fuse the whole dot product but it wedges the
  exec unit on this runtime -- NRT_EXEC_UNIT_UNRECOVERABLE.)

  Startup: DMA priority order is block-0's critical path (eT row-block,
  gT, cgo/ar2, first mask halves); mask DMAs are split at the piece
  boundary so the first pack only waits on its own half. Piece-0 and
  piece-1 matmuls use separate SBUF copies of eT so the scheduler's
  LDWEIGHTS affinity cannot pull all 8 data matmuls ahead of piece-0's
  rank-2 finishers (which would delay the first PSUM->SBUF copy).

Per core: 512 anchors x 4096 labels, 4 blocks of 128 anchors.
Output per core: [P, 8] = (masked per-anchor loss | validity); host sums
and divides.
"""

import numpy as np
import ml_dtypes

import concourse.bass as bass
import concourse.mybir as mybir
import concourse.tile as tile
from concourse import bacc
from concourse.bass_utils import run_bass_kernel_spmd
import concourse.dve_ops as dve_ops
from concourse.dve_ops import DveOp
from concourse.dve_spec import (Spec, Src0, Src1, Idx, Zero, maxx,
                                lower, _has_src1)
from concourse.dve_spec import C0 as DVE_C0, C1 as DVE_C1, C2 as DVE_C2
from concourse.dve_uop import DveOpSpec

_OPNAME = "PACK_IDX_RMAX_ANT"


def _ref_pack_idx_rmax(in0, in1, s0, s1, imm2):
    t = (in0.astype(np.float32) * in1).astype(np.float32)
    q = np.float32(np.float32(t + s1) - s1)          # round-to-nearest int
    q = q.reshape(q.shape[0], -1)
    p = np.float32(q * np.float32(imm2)
                   + np.arange(q.shape[-1], dtype=np.float32)[None, :])
    mx = np.maximum(np.float32(0.0), p.max(axis=-1, keepdims=True))
    return p.reshape(in0.shape), mx.astype(np.float32)


def register_pack_idx_rmax():
    """Custom DVE op: p[k] = round(in0[k]*in1[k])*imm2 + k,
    accum_out = max(0, row-max(p)).

    One pass fuses the mask multiply, integer quantization (s1 = 2^23
    round trick), and packing of (quantized value, element index) into one
    exactly-representable f32 (imm2 = 4096 shift), max-reduced. Values are
    arranged (C0 offset / KB flip) so every valid q is in [2048, 4095]:
    the packed value then sits in [2^23, 2^24) and the winning index is
    the accum's low 12 mantissa bits by direct bitcast -- masked-out
    entries pack to bare idx < 4096, far below any valid value, which
    doubles as the validity test. s0 is unused.
    """
    if _OPNAME in dve_ops._SUB_OPCODE_FOR_NAME:
        for op in dve_ops.OPS:
            if op.name == _OPNAME:
                return op
    spec = Spec(body=((Src0 * Src1 + DVE_C1) - DVE_C1) * DVE_C2 + Idx,
                accum=maxx, accum_init=Zero, reference=_ref_pack_idx_rmax)
    row = max(dve_ops._SUB_OPCODE_FOR_NAME.values()) + 1
    assert row < 0x20
    shas = {}
    for ver in ("v3", "v4"):
        try:
            s = DveOpSpec(name=_OPNAME, opcode=row, uops=lower(spec, ver=ver),
                          rd1_en=_has_src1(spec))
            shas[ver] = s.sha(ver)
        except Exception:
            pass
    assert shas, "PACK_IDX_RMAX_ANT failed to lower"
    op = DveOp(_OPNAME, spec, subdim=False, uops_sha=shas)
    dve_ops.OPS.append(op)
    dve_ops.CUSTOM_DVE_SPECS[_OPNAME] = spec
    dve_ops._SUB_OPCODE_FOR_NAME[_OPNAME] = row
    return op


B, D = 4096, 128
M = 8              # cores
BL = B // M        # 512 anchors per core
P = 128            # partition block
NB = BL // P       # 4 anchor blocks per core
CH = 512           # matmul chunk (one psum bank of f32)
HB = 2048          # piece width (half row, 4 psum banks)
EPS = 1e-6
C0 = 1024.0        # pins valid pos q = 2*(C0+d2) into [2048, ~2950]
KB = 4000.0 + 2.0 * C0  # neg flip: q = round(KB - 2*d2') in [~3100, 4000]
MARGIN = 1.0

F32 = mybir.dt.float32
BF16 = mybir.dt.bfloat16
U8 = mybir.dt.uint8
U32 = mybir.dt.uint32
TWO23 = float(2.0 ** 23)
PACK = 4096.0
VTH = 1.0e6        # valid rows pack >= 2^23; masked-out rows <= 4095


def build_nc(debug: bool = False):
    pack_op = register_pack_idx_rmax()
    nc = bacc.Bacc("TRN2", target_bir_lowering=False, debug=debug)

    eT = nc.dram_tensor("eT", [P, BL], BF16, kind="ExternalInput")      # -4*e_local^T
    gT = nc.dram_tensor("gT", [P, B], BF16, kind="ExternalInput")       # g^T
    cgo = nc.dram_tensor("cgo", [2, B], BF16, kind="ExternalInput")     # [2*cg ; ones]
    ar2 = nc.dram_tensor("ar2", [2, BL], BF16, kind="ExternalInput")    # [ones ; 2*arow]
    el = nc.dram_tensor("el", [P, NB, D], F32, kind="ExternalInput")    # anchor rows f32
    gfull = nc.dram_tensor("gfull", [B, D], F32, kind="ExternalInput")  # gather source
    mp = nc.dram_tensor("mp", [BL, B], U8, kind="ExternalInput")        # pos mask {0,1}
    wn = nc.dram_tensor("wn", [BL, B], U8, kind="ExternalInput")        # neg mask {0,1}

    outd = nc.dram_tensor("out", [P, 2 * NB], F32, kind="ExternalOutput")

    with tile.TileContext(nc) as tc:
        with (
            tc.tile_pool(name="singles", bufs=1) as singles,
            tc.tile_pool(name="masks", bufs=3) as maskpool,
            tc.tile_pool(name="stage", bufs=2) as stagepool,
            tc.tile_pool(name="psum", bufs=2, space="PSUM") as psumpool,
        ):
            # ---- startup DMAs, strict priority: piece-0 of block 0 first
            # (its matmuls + pos pack gate everything), then piece 1
            eTa_s = singles.tile([P, BL], BF16)    # piece-0 weights
            nc.sync.dma_start(eTa_s[:, 0:P], eT[:, 0:P])
            gT_s = singles.tile([P, B], BF16)
            for c in range(HB // CH):
                cs = slice(c * CH, (c + 1) * CH)
                nc.sync.dma_start(gT_s[:, cs], gT[:, cs])
            cgo_s = singles.tile([2, B], BF16)
            nc.sync.dma_start(cgo_s[:], cgo[:])
            ar2_s = singles.tile([2, BL], BF16)
            nc.sync.dma_start(ar2_s[:], ar2[:])
            # piece-1 matmul feed interleaved with block-0 mask halves in
            # the order the pack stream consumes them
            eTb_s = singles.tile([P, BL], BF16)    # piece-1 weights (copy)
            nc.sync.dma_start(eTb_s[:, 0:P], eT[:, 0:P])
            mp_t = [None] * NB
            wn_t = [None] * NB
            mp_t[0] = maskpool.tile([P, B], U8, tag="mp", name="mp_b0")
            wn_t[0] = maskpool.tile([P, B], U8, tag="wn", name="wn_b0")
            nc.sync.dma_start(mp_t[0][:, 0:HB], mp[0:P, 0:HB])
            for c in range(HB // CH, B // CH):
                cs = slice(c * CH, (c + 1) * CH)
                nc.sync.dma_start(gT_s[:, cs], gT[:, cs])
            nc.sync.dma_start(mp_t[0][:, HB:B], mp[0:P, HB:B])
            nc.sync.dma_start(wn_t[0][:, 0:HB], wn[0:P, 0:HB])
            nc.sync.dma_start(wn_t[0][:, HB:B], wn[0:P, HB:B])
            # non-critical startup loads
            for b in range(1, NB):
                bs = slice(b * P, (b + 1) * P)
                nc.sync.dma_start(eTa_s[:, bs], eT[:, bs])
                nc.sync.dma_start(eTb_s[:, bs], eT[:, bs])

            # ACT table warm-up in the DMA shadow (Square/Sqrt used in the
            # tail; Copy is table-free)
            warm = singles.tile([P, 1], F32)
            nc.vector.memset(warm[:], 1.0)
            nc.scalar.activation(warm[:], warm[:],
                                 mybir.ActivationFunctionType.Square)
            nc.scalar.activation(warm[:], warm[:],
                                 mybir.ActivationFunctionType.Sqrt)

            eps_b = singles.tile([P, 1], F32)
            nc.vector.memset(eps_b[:], EPS)
            m4095 = singles.tile([P, 1], U32)
            nc.vector.memset(m4095[:], 4095)

            el_all = singles.tile([P, NB, D], F32)
            nc.scalar.dma_start(el_all[:], el[:])
            acc2 = singles.tile([P, NB, 4], F32)  # per-piece accums [p0 p1 n0 n1]
            acc_s = singles.tile([P, 2], F32)     # block-0 sub-piece accums
            accP = singles.tile([P, NB], F32)     # merged packed accums
            accN = singles.tile([P, NB], F32)
            idx_pn = singles.tile([P, NB, 2], U32)
            pn_all = singles.tile([P, NB, 2, D], F32)
            v = singles.tile([P, B], F32)         # pack elementwise out (unused)
            dif = singles.tile([P, NB, 3, D], F32)
            sq = singles.tile([P, NB, 3, D], F32)
            rt2 = singles.tile([P, NB, 3], F32)   # squared distances
            rt = singles.tile([P, NB, 3], F32)    # distances
            outt = singles.tile([P, 2 * NB], F32)

            def emit_block(b):
                rs = b * P
                if b + 1 < NB:
                    # prefetch next block's masks, piece-halves first
                    nrs = (b + 1) * P
                    mp_t[b + 1] = maskpool.tile([P, B], U8, tag="mp",
                                                name=f"mp_b{b + 1}")
                    wn_t[b + 1] = maskpool.tile([P, B], U8, tag="wn",
                                                name=f"wn_b{b + 1}")
                    for h in range(2):
                        hs = slice(h * HB, (h + 1) * HB)
                        nc.sync.dma_start(mp_t[b + 1][:, hs], mp[nrs:nrs + P, hs])
                        nc.sync.dma_start(wn_t[b + 1][:, hs], wn[nrs:nrs + P, hs])

                dng = [None, None]
                for pi, lhs in ((0, eTa_s), (1, eTb_s)):
                    hs0 = pi * HB
                    psum = psumpool.tile([P, HB], F32, tag="psum",
                                         name=f"ps_{b}_{pi}")
                    for c in range(HB // CH):
                        gs = slice(hs0 + c * CH, hs0 + (c + 1) * CH)
                        ps = slice(c * CH, (c + 1) * CH)
                        nc.tensor.matmul(
                            psum[:, ps], lhsT=lhs[:, rs:rs + P],
                            rhs=gT_s[:, gs], start=True, stop=False,
                        )
                    for c in range(HB // CH):
                        gs = slice(hs0 + c * CH, hs0 + (c + 1) * CH)
                        ps = slice(c * CH, (c + 1) * CH)
                        nc.tensor.matmul(
                            psum[:, ps], lhsT=ar2_s[:, rs:rs + P],
                            rhs=cgo_s[:, gs], start=False, stop=True,
                        )
                    # neg staging = flipped values; pos packs read PSUM raw
                    dng[pi] = stagepool.tile([P, HB], F32, tag="dng",
                                             name=f"dng_{b}_{pi}")
                    nc.scalar.activation(
                        dng[pi][:], psum[:, 0:HB],
                        mybir.ActivationFunctionType.Copy,
                        bias=KB, scale=-1.0)

                    # hardest-pos pack straight off PSUM (releases it after
                    # one pack; the ACT flip above is the only other reader)
                    nc.vector._custom_dve(
                        pack_op, out=v[:, hs0:hs0 + HB], in0=psum[:, 0:HB],
                        in1=mp_t[b][:, hs0:hs0 + HB], s0=0.0, s1=TWO23,
                        imm2=PACK, accum_out=acc2[:, b, pi:pi + 1])

                # pos decode right after the pos packs: the p-gather then
                # overlaps both neg packs (the ~2.6us DGE dispatch latency
                # means later would miss the window on the last block)
                nc.vector.tensor_scalar(
                    acc2[:, b, 1:2], acc2[:, b, 1:2], 2048.0,
                    scalar2=None, op0=mybir.AluOpType.add)
                nc.vector.tensor_tensor(
                    out=accP[:, b:b + 1], in0=acc2[:, b, 0:1],
                    in1=acc2[:, b, 1:2], op=mybir.AluOpType.max)
                nc.vector.tensor_tensor(
                    out=idx_pn[:, b, 0:1], in0=accP[:, b:b + 1].bitcast(U32),
                    in1=m4095[:, 0:1], op=mybir.AluOpType.bitwise_and)
                nc.gpsimd.indirect_dma_start(
                    out=pn_all[:, b, 0, :], out_offset=None, in_=gfull[:],
                    in_offset=bass.IndirectOffsetOnAxis(
                        ap=idx_pn[:, b, 0:1], axis=0),
                )
                nc.vector._custom_dve(
                    pack_op, out=v[:, 0:HB], in0=dng[0][:],
                    in1=wn_t[b][:, 0:HB], s0=0.0, s1=TWO23,
                    imm2=PACK, accum_out=acc2[:, b, 2:3])
                nc.vector._custom_dve(
                    pack_op, out=v[:, HB:B], in0=dng[1][:],
                    in1=wn_t[b][:, HB:B], s0=0.0, s1=TWO23,
                    imm2=PACK, accum_out=acc2[:, b, 3:4])
                nc.vector.tensor_scalar(
                    acc2[:, b, 3:4], acc2[:, b, 3:4], 2048.0,
                    scalar2=None, op0=mybir.AluOpType.add)
                nc.vector.tensor_tensor(
                    out=accN[:, b:b + 1], in0=acc2[:, b, 2:3],
                    in1=acc2[:, b, 3:4], op=mybir.AluOpType.max)
                nc.vector.tensor_tensor(
                    out=idx_pn[:, b, 1:2], in0=accN[:, b:b + 1].bitcast(U32),
                    in1=m4095[:, 0:1], op=mybir.AluOpType.bitwise_and)
                nc.gpsimd.indirect_dma_start(
                    out=pn_all[:, b, 1, :], out_offset=None, in_=gfull[:],
                    in_offset=bass.IndirectOffsetOnAxis(
                        ap=idx_pn[:, b, 1:2], axis=0),
                )

            def emit_tail_vec(b):
                # exact difference rows (a-p, a-n, p-n)
                nc.gpsimd.tensor_sub(dif[:, b, 0, :], el_all[:, b, :],
                                     pn_all[:, b, 0, :])
                nc.gpsimd.tensor_sub(dif[:, b, 1, :], el_all[:, b, :],
                                     pn_all[:, b, 1, :])
                nc.gpsimd.tensor_sub(dif[:, b, 2, :], pn_all[:, b, 0, :],
                                     pn_all[:, b, 1, :])

            def emit_tail_act(b):
                # squared distance = ACT Square with fused sum-accum,
                # eps rides the bias: sum((dif + eps)^2); then sqrt
                for k in range(3):
                    nc.scalar.activation(
                        sq[:, b, k, :], dif[:, b, k, :],
                        mybir.ActivationFunctionType.Square,
                        bias=eps_b[:, 0:1], scale=1.0,
                        accum_out=rt2[:, b, k:k + 1])
                nc.scalar.activation(rt[:, b, :], rt2[:, b, :],
                                     mybir.ActivationFunctionType.Sqrt)

            # pack stream with tail subtracts lagging one block so Vector
            # never stalls on gathers; ACT tail work queued behind the
            # flips it must not delay
            emit_block(0)
            emit_block(1)
            emit_tail_vec(0)
            emit_block(2)
            emit_tail_act(0)
            emit_tail_vec(1)
            emit_block(3)
            emit_tail_act(1)
            emit_tail_vec(2)
            emit_tail_act(2)
            # validity from the merged accums (invalid rows accumulate low);
            # runs while the last gathers are in flight
            vp = singles.tile([P, NB], F32)
            vn = singles.tile([P, NB], F32)
            nc.vector.tensor_scalar(vp[:], accP[:], VTH, scalar2=None,
                                    op0=mybir.AluOpType.is_gt)
            nc.vector.tensor_scalar(vn[:], accN[:], VTH, scalar2=None,
                                    op0=mybir.AluOpType.is_gt)
            nc.vector.tensor_mul(outt[:, NB:2 * NB], vp[:], vn[:])
            emit_tail_vec(3)
            emit_tail_act(3)

            # loss = max(ap - min(an, pn) + margin, 0) * valid
            mn2 = singles.tile([P, NB], F32)
            nc.vector.tensor_tensor(out=mn2[:], in0=rt[:, :, 1],
                                    in1=rt[:, :, 2], op=mybir.AluOpType.min)
            dff = singles.tile([P, NB], F32)
            nc.vector.tensor_sub(dff[:], rt[:, :, 0], mn2[:])
            lossb = singles.tile([P, NB], F32)
            nc.vector.tensor_scalar(lossb[:], dff[:], MARGIN, scalar2=0.0,
                                    op0=mybir.AluOpType.add,
                                    op1=mybir.AluOpType.max)
            nc.vector.tensor_mul(outt[:, 0:NB], lossb[:], outt[:, NB:2 * NB])
            nc.scalar.dma_start(outd[:], outt[:])

    nc.finalize()
    return nc


def make_in_maps(embedding, target_idx, positive_idxs, negative_idxs):
    e = np.asarray(embedding, np.float32)
    tid = np.asarray(target_idx, np.int64)
    pos = np.asarray(positive_idxs)
    neg = np.asarray(negative_idxs)

    inv = np.empty(B, np.int64)
    inv[tid] = np.arange(B)
    g = np.ascontiguousarray(e[inv])                       # [B, D] f32

    e64 = e.astype(np.float64)
    g64 = g.astype(np.float64)
    sq_a = (e64 * e64).sum(1)
    s_a = e64.sum(1)
    sq_g = (g64 * g64).sum(1)
    s_g = g64.sum(1)

    gT_bf = np.ascontiguousarray(g.T).astype(ml_dtypes.bfloat16)         # [D, B]
    # doubled pipeline so packed quantization is 0.5 d2-units
    cgo_np = np.ones((2, B), np.float32)
    cgo_np[0] = 2.0 * (sq_g - 2.0 * EPS * s_g)
    cgo_bf = cgo_np.astype(ml_dtypes.bfloat16)
    arow_full = np.asarray(
        2.0 * (sq_a + 2.0 * EPS * s_a + D * EPS * EPS + C0), np.float32)

    in_maps = []
    for m in range(M):
        r = slice(m * BL, (m + 1) * BL)
        # [P, NB(, D)] layouts: block index on the free axis
        el3 = np.ascontiguousarray(
            e[r].reshape(NB, P, D).transpose(1, 0, 2))
        ar2_np = np.ones((2, BL), np.float32)
        ar2_np[1] = arow_full[r]
        in_maps.append({
            "eT": np.ascontiguousarray((-4.0 * e[r].T)).astype(ml_dtypes.bfloat16),
            "gT": gT_bf,
            "cgo": cgo_bf,
            "ar2": ar2_np.astype(ml_dtypes.bfloat16),
            "el": el3,
            "gfull": g,
            "mp": np.ascontiguousarray(pos[r].astype(np.uint8)),
            "wn": np.ascontiguousarray(neg[r].astype(np.uint8)),
        })
    return in_maps


_NC_CACHE = {}


def kernel(embedding, target_idx, positive_idxs, negative_idxs):
    in_maps = make_in_maps(embedding, target_idx, positive_idxs, negative_idxs)
    if "nc" not in _NC_CACHE:
        _NC_CACHE["nc"] = build_nc(debug=False)
    nc = _NC_CACHE["nc"]
    res = run_bass_kernel_spmd(nc, in_maps, core_ids=list(range(M)))
    total_loss = np.float64(0.0)
    total_valid = np.float64(0.0)
    for r in res.results:
        o = np.asarray(r["out"], np.float64)
        total_loss += o[:, 0:NB].sum()
        total_valid += o[:, NB:2 * NB].sum()
    return np.float32(total_loss / max(total_valid, 1.0))
